# revision 1
# baseline (speedup 1.0000x reference)
"""Trainium2 Bass kernel: batched Kabsch-aligned masked MSE (mean over batch).

Math: the reference rotation R = Vh^T diag(1,1,d) U^T (SVD of H = Pc^T Qc) is
the closest rotation to H^T.  Via the Horn quaternion formulation,
t = trace(R^T H) = lam - 4*w*(v.x)/|q|^2, where lam is the largest eigenvalue
of the 4x4 Horn matrix N built from H^T and q = (w, x) its top eigenvector.
lam is found by Halley iteration on the characteristic quartic
lam^4 - 2*trK*lam^2 - 8*detH*lam + (2*trK2 - trK^2), K = H^T H; the needed
eigenvector components come from the adjugate of (N - lam I) (rank-1 = a q q^T):
row 0 gives w*x_i, the diagonal trace gives |q|^2.
Per-sample loss = (|Pc|^2 + |Qc|^2 - 2 t) / (3 n).

Layout: batch sharded over 8 cores (4096 samples each); on-core, samples on
partitions, 32 tiles of 128 samples; stats reduced per tile with fused
multiply+reduce ops, then the 4x4 eigenproblem is solved elementwise on
[128, 32] tiles.  Per-sample losses are DMA'd out; the host sums and divides.
"""

import os
import numpy as np

import bass_rust
import concourse.bass as bass
import concourse.tile as tile
from concourse import mybir
from concourse.bass_utils import run_bass_kernel_spmd


def _legalize_single_wait(nc):
    """The deployed walrus build allows only ONE sync-wait per instruction
    (any opcode).  Tile emits multi-wait instructions, so split every
    multi-wait into a chain of single-wait same-engine Drains followed by
    the instruction carrying the final wait."""
    moved = 0
    for fn in nc.m.functions:
        for blk in fn.blocks:
            insts = blk.instructions
            new_list = []
            for ins in insts:
                si = ins.sync_info
                ow = list(si.on_wait) if si is not None and si.on_wait else []
                if len(ow) > 1:
                    for w in ow[:-1]:
                        d = mybir.InstDrain(name=f"I-sw{moved}", ins=[],
                                            outs=[], bass_is_fusable=False)
                        d.engine = ins.engine
                        d.sync_info = bass_rust.SyncInfo(on_wait=[w],
                                                         on_update=[])
                        new_list.append(d)
                        moved += 1
                    si.on_wait = [ow[-1]]
                new_list.append(ins)
            blk.instructions[:] = new_list
    return moved

F32 = mybir.dt.float32
U8 = mybir.dt.uint8
Alu = mybir.AluOpType
Act = mybir.ActivationFunctionType

N_CORES = 8
B_FULL = 32768
N_SEQ = 128
B_CORE = B_FULL // N_CORES      # 4096
N_TILES = B_CORE // 128         # 32
HALLEY_ITERS = 3



# H block order in stats: row-major k = 3*i + j
H_ORDER = [(i, j) for i in range(3) for j in range(3)]


def _phase1(tc, pools, pred, trth, msk, st, t, L):
    nc = tc.nc
    io, work, scr = pools["io"], pools["work"], pools["scr"]
    r0, r1 = t * 128, (t + 1) * 128

    pq = io.tile([128, 6 * 128], F32, tag="pq", name="pq")
    mt = io.tile([128, 128], U8, tag="mt", name="mt")
    nc.sync.dma_start(out=pq[:, 0:3 * L], in_=pred[r0:r1, 0:3 * L])
    nc.sync.dma_start(out=pq[:, 3 * L:6 * L], in_=trth[r0:r1, 0:3 * L])
    nc.sync.dma_start(out=mt[:, 0:L], in_=msk[r0:r1, 0:L])

    # w = 1 - mask (u8 -> f32) on ACT; n comes precomputed from the host
    wt = work.tile([128, 128], F32, tag="wt", name="wt")
    nc.scalar.activation(out=wt[:, 0:L], in_=mt[:, 0:L], func=Act.Identity,
                         bias=1.0, scale=-1.0)

    # One [128, 15*L] tile holds: blocks 0..8 the products Pw_i*Qw_j
    # (row-major), blocks 9..14 the masked coords [Pw_x..Qw_z]; a single
    # 3D reduce then yields H(9) + sp(3) + sq(3) per sample.
    big = work.tile([128, 15 * 128], F32, tag="big", name="big")
    pqw0 = 9 * L
    pq_v = pq[:, 0:6 * L].rearrange("p (g n c) -> p g c n", g=2, c=3)
    pqw_v = big[:, pqw0:pqw0 + 6 * L].rearrange("p (g c n) -> p g c n",
                                                g=2, c=3)
    w_b = (wt[:, 0:L].unsqueeze(1).unsqueeze(1)
           .broadcast_to([128, 2, 3, L]))
    nc.vector.tensor_tensor(out=pqw_v, in0=pq_v, in1=w_b, op=Alu.mult)

    # spp + sqq combined (only the sum is needed downstream)
    sq_scr = scr.tile([128, 6 * 128], F32, tag="actscr", name="actscr")
    nc.scalar.activation(out=sq_scr[:, 0:6 * L],
                         in_=big[:, pqw0:pqw0 + 6 * L],
                         func=Act.Square, accum_out=st["sppqq"][:, t:t + 1])

    # products, row-major H: for each i one TT computes
    # prod[3i+j] = Pw_i * Qw_j for j=0..2 (Pw_i broadcast over j)
    qw_all = big[:, pqw0 + 3 * L:pqw0 + 6 * L].rearrange(
        "p (j n) -> p j n", n=L)
    for i in range(3):
        eng = nc.vector if i == 1 else nc.gpsimd
        pwi_b = (big[:, pqw0 + i * L:pqw0 + (i + 1) * L].unsqueeze(1)
                 .broadcast_to([128, 3, L]))
        out_v = big[:, 3 * i * L:(3 * i + 3) * L].rearrange(
            "p (j n) -> p j n", n=L)
        eng.tensor_tensor(out=out_v, in0=pwi_b, in1=qw_all, op=Alu.mult)

    big_v = big[:, 0:15 * L].rearrange("p (k n) -> p k n", n=L)
    nc.vector.tensor_reduce(out=st["HSPQ"][:, t, :], in_=big_v,
                            axis=mybir.AxisListType.X, op=Alu.add)


class P2:
    """Helper for emitting elementwise phase-2 ops on [128, NT] tiles."""

    def __init__(self, tc, pool, nt, chunk=0):
        self.nc = tc.nc
        self.pool = pool
        self.nt = nt
        self.chunk = chunk
        self.ctr = 0

    def mk(self, name=None):
        self.ctr += 1
        tag = f"ch{self.chunk}_" + (name or f"p2_{self.ctr}")
        return self.pool.tile([128, self.nt], F32, tag=tag, name=tag)

    def tt(self, a, b, op, eng=None, out=None):
        dst = out if out is not None else self.mk()
        (eng or self.nc.vector).tensor_tensor(out=dst, in0=a, in1=b, op=op)
        return dst

    def mul(self, a, b, eng=None, out=None):
        return self.tt(a, b, Alu.mult, eng, out)

    def add(self, a, b, eng=None, out=None):
        return self.tt(a, b, Alu.add, eng, out)

    def sub(self, a, b, eng=None, out=None):
        return self.tt(a, b, Alu.subtract, eng, out)

    def ts(self, a, s1, op0, s2=None, op1=Alu.bypass, eng=None, out=None):
        dst = out if out is not None else self.mk()
        (eng or self.nc.vector).tensor_scalar(
            out=dst, in0=a, scalar1=s1, scalar2=s2, op0=op0, op1=op1)
        return dst

    def recip(self, a, out=None):
        dst = out if out is not None else self.mk()
        self.nc.vector.reciprocal(out=dst, in_=a)
        return dst

    def sqrt(self, a, out=None):
        dst = out if out is not None else self.mk()
        self.nc.scalar.activation(out=dst, in_=a, func=Act.Sqrt)
        return dst

    # minor2(p, q, r, s) = p*q - r*s
    def minor2(self, p, q, r, s, eng=None):
        t1 = self.mul(p, q, eng)
        t2 = self.mul(r, s, eng)
        return self.sub(t1, t2, eng)

    # det3-style combine: x*A - y*B + z*C
    def combo3(self, x, A, y, B, z, C, eng=None):
        t1 = self.mul(x, A, eng)
        t2 = self.mul(y, B, eng)
        t3 = self.mul(z, C, eng)
        return self.add(self.sub(t1, t2, eng), t3, eng)


def _phase2(tc, p2, st, loss_out):
    """Elementwise Horn eigenproblem on [128, NT] stat tiles.

    Wide multi-dim ops fuse the per-(i,j) work: H lives as [128, (t, k)]
    with k = 3*i+j row-major, so centering / K / trK2 vectorize over k.
    """
    nc = tc.nc
    V, G = nc.vector, nc.gpsimd
    NT = p2.nt

    c0, c1 = p2.c0, p2.c1
    n = st["n"][:, c0:c1]
    sppqq = st["sppqq"][:, c0:c1]
    hs = st["HSPQ"][:, c0:c1, :]         # [128, NT, 15]: H(9)+sp(3)+sq(3)
    spq = hs[:, :, 9:15]                 # [128, NT, 6] (t, c)
    Hf = hs[:, :, 0:9]                   # [128, NT, 9] (t, k)

    invn = p2.recip(n)

    # spqn = spq * invn (broadcast over c) -> [128, NT, 6]
    spqn = p2.pool.tile([128, NT * 6], F32, tag="spqn", name="spqn")
    spqn_v = spqn[:, :].rearrange("p (t c) -> p t c", c=6)
    inb6 = invn[:, :].unsqueeze(2).broadcast_to([128, NT, 6])
    V.tensor_tensor(out=spqn_v, in0=spq[:, :, :], in1=inb6, op=Alu.mult)

    # corr = sum_c spq_c * spqn_c  (= (sum sp^2 + sum sq^2)/n)
    corrp = p2.pool.tile([128, NT * 6], F32, tag="corrp", name="corrp")
    corrp_v = corrp[:, :].rearrange("p (t c) -> p t c", c=6)
    G.tensor_tensor(out=corrp_v, in0=spq[:, :, :], in1=spqn_v, op=Alu.mult)
    corr = p2.mk("corr")
    V.tensor_reduce(out=corr, in_=corrp_v, axis=mybir.AxisListType.X,
                    op=Alu.add)
    ppqqc = p2.sub(sppqq, corr, G)

    # centered covariance: Hc[t, 3i+j] = H[t, 3i+j] - sp_i * sqn_j
    m_t = p2.pool.tile([128, NT * 9], F32, tag="m_t", name="m_t")
    m_v = m_t[:, :].rearrange("p (t i j) -> p t i j", i=3, j=3)
    sp_b = bass.AP(tensor=spq.tensor, offset=spq.offset,
                   ap=[spq.ap[0], [15, NT], [1, 3], [0, 3]])
    sqn_b = bass.AP(tensor=spqn_v.tensor, offset=spqn_v.offset + 3,
                    ap=[spqn_v.ap[0], [6, NT], [0, 3], [1, 3]])
    V.tensor_tensor(out=m_v, in0=sp_b, in1=sqn_b, op=Alu.mult)
    Hc = p2.pool.tile([128, NT * 9], F32, tag="Hc", name="Hc")
    Hf_flat = bass.AP(tensor=hs.tensor, offset=hs.offset,
                      ap=[hs.ap[0], [15, NT], [1, 9]])
    V.tensor_tensor(out=Hc[:, :].rearrange("p (t k) -> p t k", k=9),
                    in0=Hf_flat, in1=m_t[:, :].rearrange(
                        "p (t k) -> p t k", k=9), op=Alu.subtract)
    Hc3 = Hc[:, :].rearrange("p (t k) -> p t k", k=9)
    h = {(i, j): Hc3[:, :, 3 * i + j] for i in range(3) for j in range(3)}

    # K[a,b] = sum_i Hc_ia * Hc_ib: per a, one product TT (t, b, i) +
    # one X reduce writing into Kt laid out (t, a, b).
    Kt = p2.pool.tile([128, NT * 9], F32, tag=f"ch{p2.chunk}_Kt", name="Kt")
    kp = p2.pool.tile([128, NT * 9], F32, tag=f"ch{p2.chunk}_kp", name="kp")
    hc0 = Hc[:, :]
    for a in range(3):
        in0 = bass.AP(tensor=Hc.tensor, offset=hc0.offset + a,
                      ap=[hc0.ap[0], [9, NT], [0, 3], [3, 3]])
        in1 = bass.AP(tensor=Hc.tensor, offset=hc0.offset,
                      ap=[hc0.ap[0], [9, NT], [1, 3], [3, 3]])
        kp_v = kp[:, :].rearrange("p (t b i) -> p t b i", b=3, i=3)
        V.tensor_tensor(out=kp_v, in0=in0, in1=in1, op=Alu.mult)
        ka_v = bass.AP(tensor=Kt.tensor, offset=Kt[:, :].offset + 3 * a,
                       ap=[Kt[:, :].ap[0], [9, NT], [1, 3]])
        V.tensor_reduce(out=ka_v, in_=kp_v, axis=mybir.AxisListType.X,
                        op=Alu.add)
    Kt3 = Kt[:, :].rearrange("p (t ab) -> p t ab", ab=9)
    K = {(a, b): Kt3[:, :, 3 * a + b] for a in range(3) for b in range(3)}

    # trK via one strided diag reduce (entries 0, 4, 8)
    trK = p2.mk("trK")
    diag_v = bass.AP(tensor=Kt.tensor, offset=Kt[:, :].offset,
                     ap=[Kt[:, :].ap[0], [9, NT], [4, 3]])
    V.tensor_reduce(out=trK, in_=diag_v, axis=mybir.AxisListType.X,
                    op=Alu.add)
    # trK2 = sum_ab K_ab^2 (all 9 entries; off-diag double-count wanted)
    k2 = p2.pool.tile([128, NT * 9], F32, tag=f"ch{p2.chunk}_k2", name="k2")
    V.tensor_tensor(out=k2[:, :], in0=Kt[:, :], in1=Kt[:, :], op=Alu.mult)
    trK2 = p2.mk("trK2")
    V.tensor_reduce(out=trK2,
                    in_=k2[:, :].rearrange("p (t ab) -> p t ab", ab=9),
                    axis=mybir.AxisListType.X, op=Alu.add)

    # detH via 2x2 minors (GpSimd, runs alongside the K block)
    m1 = p2.minor2(h[(1, 1)], h[(2, 2)], h[(1, 2)], h[(2, 1)], G)
    m2 = p2.minor2(h[(1, 0)], h[(2, 2)], h[(1, 2)], h[(2, 0)], G)
    m3 = p2.minor2(h[(1, 0)], h[(2, 1)], h[(1, 1)], h[(2, 0)], G)
    detH = p2.combo3(h[(0, 0)], m1, h[(0, 1)], m2, h[(0, 2)], m3, G)

    # quartic coefficients
    c2 = p2.ts(trK, -2.0, Alu.mult, eng=V)
    c1 = p2.ts(detH, -8.0, Alu.mult, eng=G)
    c0 = p2.sub(p2.ts(trK2, 2.0, Alu.mult, eng=V), p2.mul(trK, trK, V), V)
    c2x2 = p2.ts(c2, 2.0, Alu.mult, eng=V)

    # lam0 = sqrt(trK + 2*sqrt(3*max(E2,0))), E2 = (trK^2 - trK2)/2
    e2 = p2.sub(p2.mul(trK, trK, V), trK2, V)
    e2 = p2.ts(e2, 1.5, Alu.mult, s2=0.0, op1=Alu.max, eng=V)
    r1 = p2.sqrt(e2)
    lam = p2.sqrt(p2.tt(p2.ts(r1, 2.0, Alu.mult, eng=V), trK, Alu.add, V))

    # Halley iterations on p(l) = l^4 + c2 l^2 + c1 l + c0
    for _ in range(HALLEY_ITERS):
        lam2 = p2.mul(lam, lam, V)
        t1 = p2.add(lam2, c2, V)
        t2 = p2.mul(t1, lam2, V)
        t3 = p2.mul(c1, lam, G)
        t4 = p2.add(t3, c0, G)
        pv = p2.add(t2, t4, V)
        b1 = p2.ts(lam2, 4.0, Alu.mult, eng=G)
        b2 = p2.add(b1, c2x2, G)
        pd = p2.add(p2.mul(b2, lam, G), c1, G)
        pdd = p2.ts(lam2, 12.0, Alu.mult, eng=V)
        pdd = p2.add(pdd, c2x2, V)
        d1 = p2.mul(pd, pd, G)
        d2 = p2.ts(d1, 2.0, Alu.mult, eng=G)
        d3 = p2.mul(pv, pdd, V)
        denom = p2.sub(d2, d3, V)
        num = p2.mul(pv, pd, V)
        num = p2.ts(num, 2.0, Alu.mult, eng=V)
        rden = p2.recip(denom)
        delta = p2.mul(num, rden, V)
        lam = p2.sub(lam, delta, V)

    # one Newton polish (cheaper than a 4th Halley step, better tail)
    lam2 = p2.mul(lam, lam, V)
    t1 = p2.add(lam2, c2, V)
    t2 = p2.mul(t1, lam2, V)
    t3 = p2.mul(c1, lam, G)
    t4 = p2.add(t3, c0, G)
    pv = p2.add(t2, t4, V)
    b1 = p2.ts(lam2, 4.0, Alu.mult, eng=G)
    b2 = p2.add(b1, c2x2, G)
    pd = p2.add(p2.mul(b2, lam, G), c1, G)
    rpd = p2.recip(pd)
    lam = p2.sub(lam, p2.mul(pv, rpd, V), V)

    # Horn matrix entries (for M = Hc^T)
    n00 = p2.add(p2.add(h[(0, 0)], h[(1, 1)], G), h[(2, 2)], G)
    n01 = p2.sub(h[(2, 1)], h[(1, 2)], G)
    n02 = p2.sub(h[(0, 2)], h[(2, 0)], G)
    n03 = p2.sub(h[(1, 0)], h[(0, 1)], G)
    n11 = p2.sub(p2.sub(h[(0, 0)], h[(1, 1)], G), h[(2, 2)], G)
    n12 = p2.add(h[(1, 0)], h[(0, 1)], G)
    n13 = p2.add(h[(0, 2)], h[(2, 0)], G)
    n22 = p2.sub(p2.sub(h[(1, 1)], h[(0, 0)], G), h[(2, 2)], G)
    n23 = p2.add(h[(2, 1)], h[(1, 2)], G)
    n33 = p2.sub(p2.sub(h[(2, 2)], h[(0, 0)], G), h[(1, 1)], G)

    g00 = p2.sub(n00, lam, V)
    g11 = p2.sub(n11, lam, V)
    g22 = p2.sub(n22, lam, G)
    g33 = p2.sub(n33, lam, G)
    g01, g02, g03 = n01, n02, n03
    g12, g13, g23 = n12, n13, n23

    # 2x2 minors: D from rows (2,3), E from rows (0,1)
    D01 = p2.minor2(g02, g13, g12, g03, V)
    D02 = p2.minor2(g02, g23, g22, g03, V)
    D03 = p2.minor2(g02, g33, g23, g03, V)
    D12 = p2.minor2(g12, g23, g22, g13, G)
    D13 = p2.minor2(g12, g33, g23, g13, G)
    D23 = p2.minor2(g22, g33, g23, g23, G)
    E01 = p2.minor2(g00, g11, g01, g01, V)
    E02 = p2.minor2(g00, g12, g01, g02, V)
    E03 = p2.minor2(g00, g13, g01, g03, G)
    E12 = p2.minor2(g01, g12, g11, g02, V)
    E13 = p2.minor2(g01, g13, g11, g03, G)

    a00 = p2.combo3(g11, D23, g12, D13, g13, D12, V)
    a01n = p2.combo3(g01, D23, g02, D13, g03, D12, V)   # = -a01
    a02 = p2.combo3(g01, D13, g11, D03, g13, D01, V)
    a03n = p2.combo3(g01, D12, g11, D02, g12, D01, G)   # = -a03
    a11 = p2.combo3(g00, D23, g02, D03, g03, D02, G)
    a22 = p2.combo3(g03, E13, g13, E03, g33, E01, G)
    a33 = p2.combo3(g02, E12, g12, E02, g22, E01, V)

    tr_adj = p2.add(p2.add(a00, a11, V), p2.add(a22, a33, G), V)
    w1 = p2.mul(a01n, n01, V)
    w2 = p2.mul(a02, n02, G)
    w3 = p2.mul(a03n, n03, V)
    wx_v = p2.sub(p2.sub(w2, w1, V), w3, V)

    # t = lam - 4*wx_v/tr_adj
    rtr = p2.recip(tr_adj)
    corr2 = p2.ts(p2.mul(wx_v, rtr, V), 4.0, Alu.mult, eng=V)
    tv = p2.sub(lam, corr2, V)

    # loss = (ppc + qqc - 2 t) * invn / 3
    tot = p2.sub(ppqqc, p2.ts(tv, 2.0, Alu.mult, eng=V), V)
    lossv = p2.mul(tot, invn, V)
    p2.ts(lossv, 1.0 / 3.0, Alu.mult, eng=V, out=loss_out)


def build_program(n_tiles=N_TILES, debug_stats=False, lmax=None):
    if lmax is None:
        lmax = (N_SEQ,) * n_tiles
    assert len(lmax) == n_tiles
    nc = bass.Bass("TRN2", debug=False, enable_asserts=False,
                   target_bir_lowering=False)
    b = n_tiles * 128
    pred = nc.dram_tensor("pred", [b, 384], F32, kind="ExternalInput").ap()
    trth = nc.dram_tensor("truec", [b, 384], F32, kind="ExternalInput").ap()
    msk = nc.dram_tensor("mask", [b, 128], U8, kind="ExternalInput").ap()
    nvec = nc.dram_tensor("nvec", [128, n_tiles], F32,
                          kind="ExternalInput").ap()
    loss = nc.dram_tensor("loss", [128, n_tiles], F32,
                          kind="ExternalOutput").ap()
    dbg = {}
    if debug_stats:
        for name, shape in [("d_n", [128, n_tiles]),
                            ("d_sppqq", [128, n_tiles]),
                            ("d_spq", [128, n_tiles, 6]),
                            ("d_H", [128, n_tiles, 9])]:
            dbg[name] = nc.dram_tensor(name, shape, F32,
                                       kind="ExternalOutput").ap()

    with tile.TileContext(nc) as tc:
        from contextlib import ExitStack
        with ExitStack() as ctx:
            pools = {
                "io": ctx.enter_context(tc.tile_pool(name="io", bufs=4)),
                "work": ctx.enter_context(tc.tile_pool(name="work", bufs=4)),
                "scr": ctx.enter_context(tc.tile_pool(name="scr", bufs=4)),
                "stats": ctx.enter_context(tc.tile_pool(name="stats", bufs=1)),
                "ph2": ctx.enter_context(tc.tile_pool(name="ph2", bufs=1)),
            }
            stats = pools["stats"]
            st = {
                "n": stats.tile([128, n_tiles], F32, tag="st_n",
                                name="st_n"),
                "sppqq": stats.tile([128, n_tiles], F32, tag="st_sppqq",
                                    name="st_sppqq"),
                "HSPQ": stats.tile([128, n_tiles, 15], F32, tag="st_hspq",
                                   name="st_hspq"),
            }
            nc.sync.dma_start(out=st["n"][:, :], in_=nvec)
            loss_tile = pools["ph2"].tile([128, n_tiles], F32, tag="loss",
                                          name="loss")
            chunks = [(0, n_tiles)]
            ci = 0
            for t in range(n_tiles):
                _phase1(tc, pools, pred, trth, msk, st, t, int(lmax[t]))
                if ci < len(chunks) and t == chunks[ci][1] - 1:
                    a, b2 = chunks[ci]
                    p2 = P2(tc, pools["ph2"], b2 - a, chunk=ci)
                    p2.c0, p2.c1 = a, b2
                    _phase2(tc, p2, st, loss_tile[:, a:b2])
                    ci += 1
            nc.sync.dma_start(out=loss, in_=loss_tile[:, :])

            if debug_stats:
                nc.sync.dma_start(out=dbg["d_n"], in_=st["n"][:, :])
                nc.sync.dma_start(out=dbg["d_sppqq"], in_=st["sppqq"][:, :])
                nc.sync.dma_start(out=dbg["d_spq"], in_=st["spq"][:, :, :])
                nc.sync.dma_start(out=dbg["d_H"], in_=st["H"][:, :, :])
    _legalize_single_wait(nc)
    return nc


_nc_cache = {}


def _get_program(n_tiles=N_TILES, debug_stats=False, lmax=None):
    key = (n_tiles, debug_stats, lmax)
    if key not in _nc_cache:
        _nc_cache[key] = build_program(n_tiles, debug_stats, lmax)
    return _nc_cache[key]


def kernel(pred_coord, true_coord, pad_mask):
    """Full-input entry point: shards over 8 cores, returns scalar f32 loss.

    Samples are sorted by valid length and striped across cores so that
    tile t on every core only contains samples of length <= L_t; all
    per-tile op extents crop to L_t (~2x average saving for ragged input).
    """
    P = np.ascontiguousarray(np.asarray(pred_coord, dtype=np.float32))
    Q = np.ascontiguousarray(np.asarray(true_coord, dtype=np.float32))
    M = np.ascontiguousarray(np.asarray(pad_mask).astype(np.uint8))
    B = P.shape[0]
    assert B == B_FULL and P.shape[1] == N_SEQ

    lengths = (N_SEQ - M.sum(axis=1)).astype(np.int64)
    order = np.argsort(lengths, kind="stable")
    # global sorted position i -> core i % 8, slot i // 8; tile t on every
    # core draws from global block [1024*t, 1024*(t+1)).
    lsort = lengths[order]
    lmax = tuple(
        max(3, int(lsort[1024 * (t + 1) - 1])) for t in range(N_TILES)
    )

    P2d = P.reshape(B, 384)[order]
    Q2d = Q.reshape(B, 384)[order]
    M2d = M[order]

    nc = _get_program(lmax=lmax)
    lsrt = lengths[order].astype(np.float32)
    in_maps = []
    for c in range(N_CORES):
        ncore = lsrt[c::N_CORES]                      # [4096] slot order
        nmat = np.ascontiguousarray(
            ncore.reshape(N_TILES, 128).T)            # [128, NT]
        in_maps.append({
            "pred": np.ascontiguousarray(P2d[c::N_CORES]),
            "truec": np.ascontiguousarray(Q2d[c::N_CORES]),
            "mask": np.ascontiguousarray(M2d[c::N_CORES]),
            "nvec": nmat,
        })
    trace = bool(int(os.environ.get("KERNEL_TRACE", "0")))
    res = run_bass_kernel_spmd(nc, in_maps, core_ids=list(range(N_CORES)),
                               trace=trace)
    if trace and res.exec_time_ns is not None:
        print(f"HW exec time: {res.exec_time_ns} ns")
        kernel.last_exec_time_ns = res.exec_time_ns
    total = 0.0
    for r in res.results:
        total += r["loss"].astype(np.float64).sum()
    return np.float32(total / B)


kernel.last_exec_time_ns = None

